# revision 9
# baseline (speedup 1.0000x reference)
"""Trainium2 Bass kernel for Memorynet (KNN-interp + 1x1-conv MLP).

Strategy: pure data parallel over batch (32 batches -> 8 cores x 4).
Per batch, per 128-token tile:
  S = 2*p1@p2.T - |p2|^2  (one K=4 fp32 matmul into PSUM, [128 tok, 512 n2])
  top-8 via DVE max / max_index  (top-3 used)
  dist_k = |p1|^2 + eps - S_k ; w_k = (1/dist_k)/Z
  gather f2[idx_k] rows (bf16) via ONE indirect DMA per 4-tile group
  recvT (feature-major) accumulated in PSUM via  g_k.T @ diag(w_k)  bf16 matmuls
MLP is feature-major bf16: out tiles = W.T chunks (lhsT) @ xT chunks (rhs);
BN+ReLU folded into ScalarE activation (per-partition scale/bias), fp32 PSUM.
Host side handles all transposes / BN folding / sharding (numpy).
"""

import sys

sys.path.insert(0, "/opt/trn_rl_repo")

import numpy as np
import ml_dtypes

import concourse.bass as bass
import concourse.bacc as bacc_mod
import concourse.mybir as mybir
from concourse.tile import TileContext
from concourse.masks import make_identity
from concourse.bass_utils import run_bass_kernel_spmd

EPS_DIST = 1e-8
EPS_BN = 1e-5
NCORES = 8
BPC = 4  # batches per core
N1, N2, C1, C2 = 2048, 512, 128, 256
CIN, H1, H2 = C1 + C2, 256, 128
NT = N1 // 128  # 16 token tiles / batch
GROUP = 4       # token tiles per MLP group (512 tokens)
NG = NT // GROUP

f32 = mybir.dt.float32
bf16 = mybir.dt.bfloat16
u32 = mybir.dt.uint32
i16 = mybir.dt.int16


def build_bass():
    nc = bacc_mod.Bacc()
    p1e = nc.declare_dram_parameter("p1e", [BPC, 4, N1], f32, isOutput=False)
    rhs4 = nc.declare_dram_parameter("rhs4", [BPC, 4, N2], f32, isOutput=False)
    p1sqr = nc.declare_dram_parameter("p1sqr", [BPC, NT, 128, 8], f32, isOutput=False)
    f1T = nc.declare_dram_parameter("f1T", [BPC, C1, N1], bf16, isOutput=False)
    f2s = [
        nc.declare_dram_parameter(f"f2_{b}", [N2, C2], bf16, isOutput=False)
        for b in range(BPC)
    ]
    W1Td = nc.declare_dram_parameter("W1T", [CIN, H1], bf16, isOutput=False)
    W2Td = nc.declare_dram_parameter("W2T", [H1, H2], bf16, isOutput=False)
    sb1d = nc.declare_dram_parameter("sb1", [H1, 2], f32, isOutput=False)
    sb2d = nc.declare_dram_parameter("sb2", [H2, 2], f32, isOutput=False)
    outT = nc.declare_dram_parameter("outT", [BPC, H2, N1], f32, isOutput=True)

    AT = mybir.ActivationFunctionType
    OP = mybir.AluOpType

    with TileContext(nc) as tc:
        with (
            tc.tile_pool(name="const", bufs=1) as cpool,
            tc.tile_pool(name="batch", bufs=2) as bpool,
            tc.tile_pool(name="grp", bufs=2) as gpool,
            tc.tile_pool(name="idxp", bufs=16) as idxpool,
            tc.tile_pool(name="gk", bufs=2) as gkpool,
            tc.tile_pool(name="diag", bufs=4) as dpool,
            tc.tile_pool(name="xg", bufs=2) as xpool,
            tc.tile_pool(name="ps_s", bufs=2, space="PSUM") as ps_s,
            tc.tile_pool(name="ps_recv", bufs=1, space="PSUM") as ps_recv,
            tc.tile_pool(name="ps_mlp", bufs=1, space="PSUM") as ps_mlp,
        ):
            # ---- constants ----
            W1T = [cpool.tile([128, H1], bf16, tag=f"w1_{k}", name=f"w1_{k}") for k in range(3)]
            for k in range(3):
                nc.sync.dma_start(out=W1T[k][:], in_=W1Td[128 * k:128 * (k + 1), :])
            W2T = [cpool.tile([128, H2], bf16, tag=f"w2_{k}", name=f"w2_{k}") for k in range(2)]
            for k in range(2):
                nc.sync.dma_start(out=W2T[k][:], in_=W2Td[128 * k:128 * (k + 1), :])
            sb1 = [cpool.tile([128, 2], f32, tag=f"sb1_{k}", name=f"sb1_{k}") for k in range(2)]
            for k in range(2):
                nc.sync.dma_start(out=sb1[k][:], in_=sb1d[128 * k:128 * (k + 1), :])
            sb2 = cpool.tile([128, 2], f32)
            nc.sync.dma_start(out=sb2[:], in_=sb2d[:, :])

            for b in range(BPC):
                f1Tb = bpool.tile([C1, N1], bf16, tag="f1Tb")
                nc.sync.dma_start(out=f1Tb[:], in_=f1T[b, :, :])
                p1eb = bpool.tile([4, N1], f32, tag="p1eb")
                nc.sync.dma_start(out=p1eb[:], in_=p1e[b, :, :])
                rhsb = bpool.tile([4, N2], f32, tag="rhsb")
                nc.sync.dma_start(out=rhsb[:], in_=rhs4[b, :, :])
                f2sb = [bpool.tile([128, C2], bf16, tag=f"f2sb{c}", name=f"f2sb{c}") for c in range(4)]
                for c in range(4):
                    nc.sync.dma_start(out=f2sb[c][:], in_=f2s[b][128 * c:128 * (c + 1), :])

                for g in range(NG):
                    p1sg = gpool.tile([128, GROUP, 8], f32, tag="p1sg")
                    nc.sync.dma_start(
                        out=p1sg[:],
                        in_=p1sqr[b, GROUP * g:GROUP * (g + 1), :, :].rearrange(
                            "t p k -> p t k"
                        ),
                    )
                    maxg = idxpool.tile([128, GROUP, 8], f32, tag="maxg")
                    idxg = idxpool.tile([128, GROUP, 8], u32, tag="idxg")
                    for t in range(GROUP):
                        tau = GROUP * g + t
                        Sp = ps_s.tile([128, N2], f32, tag="Sp")
                        nc.tensor.matmul(
                            out=Sp[:],
                            lhsT=p1eb[:, 128 * tau:128 * (tau + 1)],
                            rhs=rhsb[:],
                            start=True,
                            stop=True,
                        )
                        nc.vector.max(out=maxg[:, t, :], in_=Sp[:])
                        nc.vector.max_index(
                            out=idxg[:, t, :], in_max=maxg[:, t, :], in_values=Sp[:]
                        )

                    # ---- group-batched weight math (FD=32 on DVE) ----
                    dist = gpool.tile([128, GROUP, 8], f32, tag="dist")
                    nc.vector.tensor_tensor(
                        out=dist[:], in0=p1sg[:], in1=maxg[:], op=OP.subtract
                    )
                    nc.vector.tensor_scalar_max(dist[:], dist[:], 1e-8)
                    recd = gpool.tile([128, GROUP, 8], f32, tag="recd")
                    nc.vector.reciprocal(out=recd[:], in_=dist[:])
                    Z = gpool.tile([128, GROUP], f32, tag="Z")
                    nc.vector.reduce_sum(
                        out=Z[:], in_=recd[:, :, 0:3], axis=mybir.AxisListType.X
                    )
                    Zinv = gpool.tile([128, GROUP], f32, tag="Zinv")
                    nc.vector.reciprocal(out=Zinv[:], in_=Z[:])
                    wg = gpool.tile([128, GROUP, 8], f32, tag="wg")
                    nc.vector.tensor_tensor(
                        out=wg[:],
                        in0=recd[:],
                        in1=Zinv[:, :, None].to_broadcast([128, GROUP, 8]),
                        op=OP.mult,
                    )

                    # ---- A-matrix via local_scatter, A.T via DMA xbar ----
                    wbf = gpool.tile([128, GROUP, 4], bf16, tag="wbf")
                    nc.vector.tensor_copy(out=wbf[:, :, 0:3], in_=wg[:, :, 0:3])
                    nc.vector.memset(wbf[:, :, 3:4], 0.0)
                    idx16 = gpool.tile([128, GROUP, 4], i16, tag="idx16")
                    nc.vector.tensor_copy(out=idx16[:, :, 0:3], in_=idxg[:, :, 0:3])
                    nc.vector.memset(idx16[:, :, 3:4], -1)
                    ATg = [
                        gkpool.tile([128, 512], bf16, tag=f"ATg{c}", name=f"ATg{c}")
                        for c in range(4)
                    ]
                    for t in range(GROUP):
                        A = dpool.tile([128, N2], bf16, tag="A")
                        nc.gpsimd.local_scatter(
                            out_ap=A[:],
                            data_ap=wbf[:, t, :],
                            idxs_ap=idx16[:, t, :],
                            channels=128,
                            num_elems=N2,
                            num_idxs=4,
                        )
                        for c in range(4):
                            nc.sync.dma_start_transpose(
                                out=ATg[c][:, 128 * t:128 * (t + 1)],
                                in_=A[:, 128 * c:128 * (c + 1)],
                            )

                    # ---- recvT: f2 chunks (lhsT) @ A.T chunks, K=512 ----
                    recvp = [
                        ps_recv.tile([128, 512], f32, tag=f"recvp{h}", name=f"recvp{h}")
                        for h in range(2)
                    ]
                    for h in range(2):
                        for c in range(4):
                            nc.tensor.matmul(
                                out=recvp[h][:],
                                lhsT=f2sb[c][:, 128 * h:128 * (h + 1)],
                                rhs=ATg[c][:],
                                start=(c == 0),
                                stop=(c == 3),
                            )

                    # ---- xT chunks in SBUF (bf16): [recvT0, recvT1, f1T-slice] ----
                    xg = [xpool.tile([128, 512], bf16, tag=f"xg{h}", name=f"xg{h}") for h in range(2)]
                    for h in range(2):
                        nc.scalar.activation(
                            out=xg[h][:], in_=recvp[h][:], func=AT.Copy, bias=0.0
                        )
                    f1sl = f1Tb[:, 512 * g:512 * (g + 1)]

                    # ---- L1: h1T [2x128, 512] ----
                    h1 = [xpool.tile([128, 512], bf16, tag=f"h1_{m}", name=f"h1_{m}") for m in range(2)]
                    for m in range(2):
                        l1p = ps_mlp.tile([128, 512], f32, tag="l1p")
                        for kk in range(3):
                            rhs_kk = xg[kk][:] if kk < 2 else f1sl
                            nc.tensor.matmul(
                                out=l1p[:],
                                lhsT=W1T[kk][:, 128 * m:128 * (m + 1)],
                                rhs=rhs_kk,
                                start=(kk == 0),
                                stop=(kk == 2),
                            )
                        nc.scalar.activation(
                            out=h1[m][:],
                            in_=l1p[:],
                            func=AT.Relu,
                            scale=sb1[m][:, 0:1],
                            bias=sb1[m][:, 1:2],
                        )

                    # ---- L2: h2T [128, 512] ----
                    l2p = ps_mlp.tile([128, 512], f32, tag="l2p")
                    for kk in range(2):
                        nc.tensor.matmul(
                            out=l2p[:],
                            lhsT=W2T[kk][:],
                            rhs=h1[kk][:],
                            start=(kk == 0),
                            stop=(kk == 1),
                        )
                    o = xpool.tile([128, 512], f32, tag="osb")
                    nc.scalar.activation(
                        out=o[:],
                        in_=l2p[:],
                        func=AT.Relu,
                        scale=sb2[:, 0:1],
                        bias=sb2[:, 1:2],
                    )
                    nc.sync.dma_start(
                        out=outT[b, :, 512 * g:512 * (g + 1)], in_=o[:]
                    )
    nc.compile()
    return nc


_CACHE = {}


def _get_nc():
    if "nc" not in _CACHE:
        _CACHE["nc"] = build_bass()
    return _CACHE["nc"]


def _prep_core(inputs, c):
    """Host-side prep of one core's input map (batches 4c..4c+4)."""
    sl = slice(BPC * c, BPC * (c + 1))
    p1 = inputs["points_1"][sl]     # [4, N1, 3]
    p2 = inputs["points_2"][sl]     # [4, N2, 3]
    f1 = inputs["features_1"][sl]   # [4, N1, C1]
    f2 = inputs["features_2"][sl]   # [4, N2, C2]

    p1e = np.empty((BPC, 4, N1), np.float32)
    p1e[:, 0:3, :] = np.transpose(p1, (0, 2, 1))
    p1e[:, 3, :] = 1.0
    rhs4 = np.empty((BPC, 4, N2), np.float32)
    rhs4[:, 0:3, :] = 2.0 * np.transpose(p2, (0, 2, 1))
    rhs4[:, 3, :] = -np.sum(p2.astype(np.float64) ** 2, -1)
    p1sq = np.sum(p1.astype(np.float64) ** 2, -1) + EPS_DIST  # [4, N1]
    p1sqr = np.broadcast_to(
        p1sq.reshape(BPC, NT, 128, 1), (BPC, NT, 128, 8)
    ).astype(np.float32)
    m = {
        "p1e": np.ascontiguousarray(p1e),
        "rhs4": np.ascontiguousarray(rhs4.astype(np.float32)),
        "p1sqr": np.ascontiguousarray(p1sqr),
        "f1T": np.ascontiguousarray(
            np.transpose(f1, (0, 2, 1)).astype(ml_dtypes.bfloat16)
        ),
    }
    for b in range(BPC):
        m[f"f2_{b}"] = np.ascontiguousarray(f2[b].astype(ml_dtypes.bfloat16))
    # shared weights
    s1 = inputs["g1"] / np.sqrt(inputs["v1"] + EPS_BN)
    b1f = (inputs["b1"] - inputs["m1"]) * s1 + inputs["be1"]
    s2 = inputs["g2"] / np.sqrt(inputs["v2"] + EPS_BN)
    b2f = (inputs["b2"] - inputs["m2"]) * s2 + inputs["be2"]
    m["W1T"] = np.ascontiguousarray(inputs["W1"].T.astype(ml_dtypes.bfloat16))
    m["W2T"] = np.ascontiguousarray(inputs["W2"].T.astype(ml_dtypes.bfloat16))
    m["sb1"] = np.ascontiguousarray(np.stack([s1, b1f], -1).astype(np.float32))
    m["sb2"] = np.ascontiguousarray(np.stack([s2, b2f], -1).astype(np.float32))
    return m


def run(inputs, trace=False):
    nc = _get_nc()
    in_maps = [_prep_core(inputs, c) for c in range(NCORES)]
    res = run_bass_kernel_spmd(
        nc, in_maps, core_ids=list(range(NCORES)), trace=trace
    )
    outs = [np.asarray(r["outT"]) for r in res.results]
    full = np.concatenate(outs, 0)          # [32, H2, N1]
    out = np.ascontiguousarray(np.transpose(full, (0, 2, 1)))  # [32, N1, H2]
    return out, res


def kernel(**inputs):
    out, _ = run(inputs, trace=False)
    return out


# revision 10
# speedup vs baseline: 2.5239x; 2.5239x over previous
"""Trainium2 Bass kernel for Memorynet (KNN-interp + 1x1-conv MLP).

Strategy: pure data parallel over batch (32 batches -> 8 cores x 4).
Per batch, per 128-token tile:
  S = 2*p1@p2.T - |p2|^2  (one K=4 fp32 matmul into PSUM, [128 tok, 512 n2])
  top-8 via DVE max / max_index  (top-3 used)
  dist_k = |p1|^2 + eps - S_k ; w_k = (1/dist_k)/Z
  gather f2[idx_k] rows (bf16) via ONE indirect DMA per 4-tile group
  recvT (feature-major) accumulated in PSUM via  g_k.T @ diag(w_k)  bf16 matmuls
MLP is feature-major bf16: out tiles = W.T chunks (lhsT) @ xT chunks (rhs);
BN+ReLU folded into ScalarE activation (per-partition scale/bias), fp32 PSUM.
Host side handles all transposes / BN folding / sharding (numpy).
"""

import sys

sys.path.insert(0, "/opt/trn_rl_repo")

import numpy as np
import ml_dtypes

import concourse.bass as bass
import concourse.bacc as bacc_mod
import concourse.mybir as mybir
from concourse.tile import TileContext
from concourse.masks import make_identity
from concourse.bass_utils import run_bass_kernel_spmd

EPS_DIST = 1e-8
EPS_BN = 1e-5
NCORES = 8
BPC = 4  # batches per core
N1, N2, C1, C2 = 2048, 512, 128, 256
CIN, H1, H2 = C1 + C2, 256, 128
NT = N1 // 128  # 16 token tiles / batch
GROUP = 4       # token tiles per MLP group (512 tokens)
NG = NT // GROUP

f32 = mybir.dt.float32
bf16 = mybir.dt.bfloat16
u32 = mybir.dt.uint32
i16 = mybir.dt.int16


def build_bass():
    nc = bacc_mod.Bacc()
    p1e = nc.declare_dram_parameter("p1e", [BPC, 21, N1], bf16, isOutput=False)
    rhs4 = nc.declare_dram_parameter("rhs4", [BPC, 21, N2], bf16, isOutput=False)
    p1sqr = nc.declare_dram_parameter("p1sqr", [BPC, NT, 128, 8], f32, isOutput=False)
    f1T = nc.declare_dram_parameter("f1T", [BPC, C1, N1], bf16, isOutput=False)
    f2s = [
        nc.declare_dram_parameter(f"f2_{b}", [N2, C2], bf16, isOutput=False)
        for b in range(BPC)
    ]
    W1Td = nc.declare_dram_parameter("W1T", [CIN, H1], bf16, isOutput=False)
    W2Td = nc.declare_dram_parameter("W2T", [H1, H2], bf16, isOutput=False)
    sb1d = nc.declare_dram_parameter("sb1", [H1, 2], f32, isOutput=False)
    sb2d = nc.declare_dram_parameter("sb2", [H2, 2], f32, isOutput=False)
    outT = nc.declare_dram_parameter("outT", [BPC, H2, N1], f32, isOutput=True)

    AT = mybir.ActivationFunctionType
    OP = mybir.AluOpType

    with TileContext(nc) as tc:
        with (
            tc.tile_pool(name="const", bufs=1) as cpool,
            tc.tile_pool(name="batch", bufs=2) as bpool,
            tc.tile_pool(name="grp", bufs=2) as gpool,
            tc.tile_pool(name="idxp", bufs=16) as idxpool,
            tc.tile_pool(name="gk", bufs=2) as gkpool,
            tc.tile_pool(name="diag", bufs=4) as dpool,
            tc.tile_pool(name="xg", bufs=2) as xpool,
            tc.tile_pool(name="ps_s", bufs=2, space="PSUM") as ps_s,
            tc.tile_pool(name="ps_recv", bufs=1, space="PSUM") as ps_recv,
            tc.tile_pool(name="ps_mlp", bufs=1, space="PSUM") as ps_mlp,
        ):
            # ---- constants ----
            W1T = [cpool.tile([128, H1], bf16, tag=f"w1_{k}", name=f"w1_{k}") for k in range(3)]
            for k in range(3):
                nc.sync.dma_start(out=W1T[k][:], in_=W1Td[128 * k:128 * (k + 1), :])
            W2T = [cpool.tile([128, H2], bf16, tag=f"w2_{k}", name=f"w2_{k}") for k in range(2)]
            for k in range(2):
                nc.sync.dma_start(out=W2T[k][:], in_=W2Td[128 * k:128 * (k + 1), :])
            sb1 = [cpool.tile([128, 2], f32, tag=f"sb1_{k}", name=f"sb1_{k}") for k in range(2)]
            for k in range(2):
                nc.sync.dma_start(out=sb1[k][:], in_=sb1d[128 * k:128 * (k + 1), :])
            sb2 = cpool.tile([128, 2], f32)
            nc.sync.dma_start(out=sb2[:], in_=sb2d[:, :])

            for b in range(BPC):
                f1Tb = bpool.tile([C1, N1], bf16, tag="f1Tb")
                nc.sync.dma_start(out=f1Tb[:], in_=f1T[b, :, :])
                p1eb = bpool.tile([21, N1], bf16, tag="p1eb")
                nc.sync.dma_start(out=p1eb[:], in_=p1e[b, :, :])
                rhsb = bpool.tile([21, N2], bf16, tag="rhsb")
                nc.sync.dma_start(out=rhsb[:], in_=rhs4[b, :, :])
                f2sb = [bpool.tile([128, C2], bf16, tag=f"f2sb{c}", name=f"f2sb{c}") for c in range(4)]
                for c in range(4):
                    nc.sync.dma_start(out=f2sb[c][:], in_=f2s[b][128 * c:128 * (c + 1), :])

                for g in range(NG):
                    p1sg = gpool.tile([128, GROUP, 8], f32, tag="p1sg")
                    nc.sync.dma_start(
                        out=p1sg[:],
                        in_=p1sqr[b, GROUP * g:GROUP * (g + 1), :, :].rearrange(
                            "t p k -> p t k"
                        ),
                    )
                    maxg = idxpool.tile([128, GROUP, 8], f32, tag="maxg")
                    idxg = idxpool.tile([128, GROUP, 8], u32, tag="idxg")
                    for t in range(GROUP):
                        tau = GROUP * g + t
                        Sp = ps_s.tile([128, N2], f32, tag="Sp")
                        nc.tensor.matmul(
                            out=Sp[:],
                            lhsT=p1eb[:, 128 * tau:128 * (tau + 1)],
                            rhs=rhsb[:],
                            start=True,
                            stop=True,
                        )
                        nc.vector.max(out=maxg[:, t, :], in_=Sp[:])
                        nc.vector.max_index(
                            out=idxg[:, t, :], in_max=maxg[:, t, :], in_values=Sp[:]
                        )

                    # ---- group-batched weight math (FD=32 on DVE) ----
                    dist = gpool.tile([128, GROUP, 8], f32, tag="dist")
                    nc.vector.tensor_tensor(
                        out=dist[:], in0=p1sg[:], in1=maxg[:], op=OP.subtract
                    )
                    nc.vector.tensor_scalar_max(dist[:], dist[:], 1e-8)
                    recd = gpool.tile([128, GROUP, 8], f32, tag="recd")
                    nc.vector.reciprocal(out=recd[:], in_=dist[:])
                    Z = gpool.tile([128, GROUP], f32, tag="Z")
                    nc.vector.reduce_sum(
                        out=Z[:], in_=recd[:, :, 0:3], axis=mybir.AxisListType.X
                    )
                    Zinv = gpool.tile([128, GROUP], f32, tag="Zinv")
                    nc.vector.reciprocal(out=Zinv[:], in_=Z[:])
                    wg = gpool.tile([128, GROUP, 8], f32, tag="wg")
                    nc.vector.tensor_tensor(
                        out=wg[:],
                        in0=recd[:],
                        in1=Zinv[:, :, None].to_broadcast([128, GROUP, 8]),
                        op=OP.mult,
                    )

                    # ---- A-matrix via local_scatter, A.T via DMA xbar ----
                    wbf = gpool.tile([128, GROUP, 4], bf16, tag="wbf")
                    nc.vector.tensor_copy(out=wbf[:, :, 0:3], in_=wg[:, :, 0:3])
                    nc.vector.memset(wbf[:, :, 3:4], 0.0)
                    idx16 = gpool.tile([128, GROUP, 4], i16, tag="idx16")
                    nc.vector.tensor_copy(out=idx16[:, :, 0:3], in_=idxg[:, :, 0:3])
                    nc.vector.memset(idx16[:, :, 3:4], -1)
                    Ag = dpool.tile([128, GROUP, N2], bf16, tag="A")
                    for t in range(GROUP):
                        nc.gpsimd.local_scatter(
                            out_ap=Ag[:, t, :],
                            data_ap=wbf[:, t, :],
                            idxs_ap=idx16[:, t, :],
                            channels=128,
                            num_elems=N2,
                            num_idxs=4,
                        )
                    ATt = gkpool.tile([128, 16, 128], bf16, tag="ATt")
                    nc.sync.dma_start_transpose(out=ATt[:], in_=Ag[:])
                    ATv = ATt[:].rearrange("p (t c) r -> p c t r", c=4)

                    # ---- recvT: f2 chunks (lhsT) @ A.T chunks, K=512 ----
                    recvp = [
                        ps_recv.tile([128, 512], f32, tag=f"recvp{h}", name=f"recvp{h}")
                        for h in range(2)
                    ]
                    for h in range(2):
                        for c in range(4):
                            nc.tensor.matmul(
                                out=recvp[h][:],
                                lhsT=f2sb[c][:, 128 * h:128 * (h + 1)],
                                rhs=ATv[:, c],
                                start=(c == 0),
                                stop=(c == 3),
                            )

                    # ---- xT chunks in SBUF (bf16): [recvT0, recvT1, f1T-slice] ----
                    xg = [xpool.tile([128, 512], bf16, tag=f"xg{h}", name=f"xg{h}") for h in range(2)]
                    for h in range(2):
                        nc.scalar.activation(
                            out=xg[h][:], in_=recvp[h][:], func=AT.Copy, bias=0.0
                        )
                    f1sl = f1Tb[:, 512 * g:512 * (g + 1)]

                    # ---- L1: h1T [2x128, 512] ----
                    h1 = [xpool.tile([128, 512], bf16, tag=f"h1_{m}", name=f"h1_{m}") for m in range(2)]
                    for m in range(2):
                        l1p = ps_mlp.tile([128, 512], f32, tag="l1p")
                        for kk in range(3):
                            rhs_kk = xg[kk][:] if kk < 2 else f1sl
                            nc.tensor.matmul(
                                out=l1p[:],
                                lhsT=W1T[kk][:, 128 * m:128 * (m + 1)],
                                rhs=rhs_kk,
                                start=(kk == 0),
                                stop=(kk == 2),
                            )
                        nc.scalar.activation(
                            out=h1[m][:],
                            in_=l1p[:],
                            func=AT.Relu,
                            scale=sb1[m][:, 0:1],
                            bias=sb1[m][:, 1:2],
                        )

                    # ---- L2: h2T [128, 512] ----
                    l2p = ps_mlp.tile([128, 512], f32, tag="l2p")
                    for kk in range(2):
                        nc.tensor.matmul(
                            out=l2p[:],
                            lhsT=W2T[kk][:],
                            rhs=h1[kk][:],
                            start=(kk == 0),
                            stop=(kk == 1),
                        )
                    o = xpool.tile([128, 512], f32, tag="osb")
                    nc.scalar.activation(
                        out=o[:],
                        in_=l2p[:],
                        func=AT.Relu,
                        scale=sb2[:, 0:1],
                        bias=sb2[:, 1:2],
                    )
                    nc.sync.dma_start(
                        out=outT[b, :, 512 * g:512 * (g + 1)], in_=o[:]
                    )
    nc.compile()
    return nc


_CACHE = {}


def _get_nc():
    if "nc" not in _CACHE:
        _CACHE["nc"] = build_bass()
    return _CACHE["nc"]


def _prep_core(inputs, c):
    """Host-side prep of one core's input map (batches 4c..4c+4)."""
    sl = slice(BPC * c, BPC * (c + 1))
    p1 = inputs["points_1"][sl]     # [4, N1, 3]
    p2 = inputs["points_2"][sl]     # [4, N2, 3]
    f1 = inputs["features_1"][sl]   # [4, N1, C1]
    f2 = inputs["features_2"][sl]   # [4, N2, C2]

    def split3(x):
        a = x.astype(ml_dtypes.bfloat16)
        r = x - a.astype(np.float32)
        bb = r.astype(ml_dtypes.bfloat16)
        cc = (r - bb.astype(np.float32)).astype(ml_dtypes.bfloat16)
        return a, bb, cc

    p1T = np.transpose(p1, (0, 2, 1)).astype(np.float32)   # [4, 3, N1]
    p2T2 = (2.0 * np.transpose(p2, (0, 2, 1))).astype(np.float32)  # [4, 3, N2]
    p2sq = np.sum(p2.astype(np.float64) ** 2, -1)          # [4, N2]
    a1, b1_, c1_ = split3(p1T)
    x2, y2, z2 = split3(p2T2)
    s1_, s2_, s3_ = split3((-p2sq).astype(np.float32))
    onesr = np.ones((BPC, 1, N1), ml_dtypes.bfloat16)
    p1e = np.concatenate(
        [a1, a1, b1_, a1, b1_, c1_, onesr, onesr, onesr], axis=1
    )  # [4, 21, N1]
    rhs4 = np.concatenate(
        [x2, y2, x2, z2, y2, x2,
         s1_[:, None, :], s2_[:, None, :], s3_[:, None, :]], axis=1
    )  # [4, 21, N2]
    p1sq = np.sum(p1.astype(np.float64) ** 2, -1) + EPS_DIST  # [4, N1]
    p1sqr = np.broadcast_to(
        p1sq.reshape(BPC, NT, 128, 1), (BPC, NT, 128, 8)
    ).astype(np.float32)
    m = {
        "p1e": np.ascontiguousarray(p1e.astype(ml_dtypes.bfloat16)),
        "rhs4": np.ascontiguousarray(rhs4.astype(ml_dtypes.bfloat16)),
        "p1sqr": np.ascontiguousarray(p1sqr),
        "f1T": np.ascontiguousarray(
            np.transpose(f1, (0, 2, 1)).astype(ml_dtypes.bfloat16)
        ),
    }
    for b in range(BPC):
        m[f"f2_{b}"] = np.ascontiguousarray(f2[b].astype(ml_dtypes.bfloat16))
    # shared weights
    s1 = inputs["g1"] / np.sqrt(inputs["v1"] + EPS_BN)
    b1f = (inputs["b1"] - inputs["m1"]) * s1 + inputs["be1"]
    s2 = inputs["g2"] / np.sqrt(inputs["v2"] + EPS_BN)
    b2f = (inputs["b2"] - inputs["m2"]) * s2 + inputs["be2"]
    m["W1T"] = np.ascontiguousarray(inputs["W1"].T.astype(ml_dtypes.bfloat16))
    m["W2T"] = np.ascontiguousarray(inputs["W2"].T.astype(ml_dtypes.bfloat16))
    m["sb1"] = np.ascontiguousarray(np.stack([s1, b1f], -1).astype(np.float32))
    m["sb2"] = np.ascontiguousarray(np.stack([s2, b2f], -1).astype(np.float32))
    return m


def run(inputs, trace=False):
    nc = _get_nc()
    in_maps = [_prep_core(inputs, c) for c in range(NCORES)]
    res = run_bass_kernel_spmd(
        nc, in_maps, core_ids=list(range(NCORES)), trace=trace
    )
    outs = [np.asarray(r["outT"]) for r in res.results]
    full = np.concatenate(outs, 0)          # [32, H2, N1]
    out = np.ascontiguousarray(np.transpose(full, (0, 2, 1)))  # [32, N1, H2]
    return out, res


def kernel(**inputs):
    out, _ = run(inputs, trace=False)
    return out
